# revision 2
# baseline (speedup 1.0000x reference)
"""Causal single-head attention on 8 TRN2 NeuronCores.

Data-parallel over batch: core b computes attention for batch element b.
Inputs are pre-transposed on the host (xT = x[b].T) so the device kernel
never has to transpose the activations; causality is exploited on-device
(only score tiles on/below the diagonal are computed; the diagonal tile is
masked with affine_select). The softmax denominator comes for free from a
ones-column appended to V in the PV matmul.
"""

import os
import sys

sys.path.insert(0, "/opt/trn_rl_repo")

import numpy as np

BS, SEQ, D, E = 8, 2048, 768, 64
P = 128                 # SBUF partitions
CHUNK = 512             # q-chunk (matmul moving free dim)
N_CHUNKS = SEQ // CHUNK  # 4
N_KT = SEQ // P          # 16 key tiles
N_DT = D // P            # 6 contraction tiles for the projections
SCALE = 1.0 / np.sqrt(E).astype(np.float32)  # 0.125

# Compute dtype for the matmul operands ("float32", "bfloat16", "float32r")
DT_MM_NAME = os.environ.get("ATTN_DT_MM", "float32")

_CACHE = {}

LAST_RESULT = None  # BassKernelResults of the most recent run (for test.py)


def _build(dt_mm_name):
    from contextlib import ExitStack

    import concourse.bass as bass  # noqa: F401  (import registers engines)
    import concourse.tile as tile
    from concourse import bacc, mybir
    from concourse.masks import make_identity

    f32 = mybir.dt.float32
    dt_mm = getattr(mybir.dt, dt_mm_name)
    # DRAM layout dtype: float32r is a matmul-operand view; store as f32.
    dt_in = f32 if dt_mm_name == "float32r" else dt_mm

    nc = bacc.Bacc(
        "TRN2", target_bir_lowering=False, debug=False, num_devices=BS
    )
    xT_d = nc.dram_tensor("xT", [D, SEQ], dt_in, kind="ExternalInput").ap()
    wq_d = nc.dram_tensor("Wq", [D, E], dt_in, kind="ExternalInput").ap()
    wk_d = nc.dram_tensor("Wk", [D, E], dt_in, kind="ExternalInput").ap()
    wv_d = nc.dram_tensor("Wv", [D, E], dt_in, kind="ExternalInput").ap()
    out_d = nc.dram_tensor("out", [SEQ, E], f32, kind="ExternalOutput").ap()

    def as_mm(ap):
        # reinterpret an f32 SBUF AP as float32r for the matmul operands
        if dt_mm_name == "float32r":
            return ap.bitcast(mybir.dt.float32r)
        return ap

    with tile.TileContext(nc) as tc, ExitStack() as ctx:
        const = ctx.enter_context(tc.tile_pool(name="const", bufs=1))
        mm_ps = ctx.enter_context(tc.tile_pool(name="mm_ps", bufs=3, space="PSUM"))
        pv_ps = ctx.enter_context(tc.tile_pool(name="pv_ps", bufs=2, space="PSUM"))
        ot_ps = ctx.enter_context(tc.tile_pool(name="ot_ps", bufs=2, space="PSUM"))
        p_pool = ctx.enter_context(tc.tile_pool(name="p_pool", bufs=4))
        o_pool = ctx.enter_context(tc.tile_pool(name="o_pool", bufs=2))

        ident = const.tile([P, P], f32)
        make_identity(nc, ident)
        if dt_in != f32:
            ident_mm = const.tile([P, P], dt_in)
            make_identity(nc, ident_mm)
        else:
            ident_mm = ident

        # --- load weights and xT ---
        w_sbs = []
        for name, w_d in (("wq", wq_d), ("wk", wk_d), ("wv", wv_d)):
            w_sb = const.tile([P, N_DT, E], dt_in, tag=name)
            for d in range(N_DT):
                nc.sync.dma_start(w_sb[:, d, :], w_d[d * P:(d + 1) * P, :])
            w_sbs.append(w_sb)
        wq_sb, wk_sb, wv_sb = w_sbs

        xT_sb = const.tile([P, N_DT, SEQ], dt_in, tag="xT")
        for d in range(N_DT):
            nc.sync.dma_start(xT_sb[:, d, :], xT_d[d * P:(d + 1) * P, :])

        # --- projections: qT/kT/vT = W.T @ xT, [64, SEQ] each ---
        qT_sb = const.tile([E, SEQ], dt_in, tag="qT")
        kT_sb = const.tile([E, SEQ], dt_in, tag="kT")
        vT_sb = const.tile([E, SEQ], dt_in, tag="vT")
        for c in range(N_CHUNKS):
            sl = slice(c * CHUNK, (c + 1) * CHUNK)
            for w_sb, dst in ((wq_sb, qT_sb), (wk_sb, kT_sb), (wv_sb, vT_sb)):
                ps = mm_ps.tile([E, CHUNK], f32, tag="mm")
                for d in range(N_DT):
                    nc.tensor.matmul(
                        ps,
                        lhsT=as_mm(w_sb[:, d, :]),
                        rhs=as_mm(xT_sb[:, d, sl]),
                        start=(d == 0),
                        stop=(d == N_DT - 1),
                    )
                nc.vector.tensor_copy(dst[:, sl], ps)

        # --- v natural [128, 16, 65]: transpose vT back, ones in col E ---
        v_sb = const.tile([P, N_KT, E + 1], dt_in, tag="v")
        nc.vector.memset(v_sb[:], 1.0)
        for t in range(N_KT):
            vt = mm_ps.tile([P, E], f32, tag="mm")
            nc.tensor.transpose(
                vt, vT_sb[:, t * P:(t + 1) * P], ident_mm[0:E, 0:E]
            )
            nc.vector.tensor_copy(v_sb[:, t, 0:E], vt)

        # --- flash loop over q-chunks ---
        for c in range(N_CHUNKS):
            sl = slice(c * CHUNK, (c + 1) * CHUNK)
            n_sk = 4 * (c + 1)  # causal: only key tiles covering k <= q
            pv = pv_ps.tile([E + 1, CHUNK], f32, tag="pv")
            for t in range(n_sk):
                s_ps = mm_ps.tile([P, CHUNK], f32, tag="mm")
                nc.tensor.matmul(
                    s_ps,
                    lhsT=as_mm(kT_sb[:, t * P:(t + 1) * P]),
                    rhs=as_mm(qT_sb[:, sl]),
                    start=True,
                    stop=True,
                )
                p_sb = p_pool.tile([P, CHUNK], dt_in, tag="p")
                nc.scalar.activation(
                    p_sb, s_ps, mybir.ActivationFunctionType.Exp, scale=float(SCALE)
                )
                if t >= 4 * c:
                    # diagonal tile: zero out entries with k > q
                    j = t - 4 * c
                    nc.gpsimd.affine_select(
                        out=p_sb,
                        in_=p_sb,
                        compare_op=mybir.AluOpType.is_ge,
                        fill=0.0,
                        base=-j * P,
                        pattern=[[1, CHUNK]],
                        channel_multiplier=-1,
                    )
                nc.tensor.matmul(
                    pv,
                    lhsT=as_mm(v_sb[:, t, :]),
                    rhs=as_mm(p_sb[:]),
                    start=(t == 0),
                    stop=(t == n_sk - 1),
                )
            # transpose [E+1, CHUNK] -> 4x [128, E+1], normalize, store
            pvT_sb = o_pool.tile([E + 1, CHUNK], f32, tag="pvT")
            nc.vector.tensor_copy(pvT_sb, pv)
            for st in range(CHUNK // P):
                ot = ot_ps.tile([P, E + 1], f32, tag="ot")
                nc.tensor.transpose(
                    ot,
                    pvT_sb[:, st * P:(st + 1) * P],
                    ident[0:E + 1, 0:E + 1],
                )
                recip = o_pool.tile([P, 1], f32, tag="recip")
                nc.vector.reciprocal(recip, ot[:, E:E + 1])
                o_sb = o_pool.tile([P, E], f32, tag="o")
                nc.vector.tensor_scalar_mul(o_sb, ot[:, 0:E], recip)
                r0 = c * CHUNK + st * P
                nc.sync.dma_start(out_d[r0:r0 + P, :], o_sb)

    nc.compile()
    return nc


def _get(dt_mm_name=None):
    name = dt_mm_name or DT_MM_NAME
    if name not in _CACHE:
        _CACHE[name] = _build(name)
    return _CACHE[name]


def _ensure_axon_hooks():
    """The agent image's antenv lacks axon_hooks; bass_utils imports it when
    trace=True under axon. Provide it, wired to the real ctypes NTFF
    profiler from trn_agent_boot when available."""
    try:
        import antenv.axon_hooks  # noqa: F401

        return
    except ImportError:
        pass
    import types

    try:
        import antenv
    except ImportError:
        return
    mod = types.ModuleType("antenv.axon_hooks")
    mod._hook = None

    def set_axon_ntff_profile_hook(h):
        mod._hook = h

    def get_axon_ntff_profile_hook():
        return mod._hook

    mod.set_axon_ntff_profile_hook = set_axon_ntff_profile_hook
    mod.get_axon_ntff_profile_hook = get_axon_ntff_profile_hook
    sys.modules["antenv.axon_hooks"] = mod
    antenv.axon_hooks = mod
    try:
        from trn_agent_boot.trn_boot import _ntff_profile_via_ctypes

        so_path = "/opt/axon/libaxon_pjrt.so"
        if os.path.exists(so_path):
            mod._hook = _ntff_profile_via_ctypes(so_path)
    except Exception:
        pass


def kernel(x, mask, Wq, Wk, Wv):
    global LAST_RESULT
    _ensure_axon_hooks()
    from concourse.bass_utils import run_bass_kernel_spmd

    nc = _get()

    if DT_MM_NAME == "bfloat16":
        import ml_dtypes

        np_dt = ml_dtypes.bfloat16
    else:
        np_dt = np.float32

    x = np.asarray(x, dtype=np.float32)
    wq = np.ascontiguousarray(np.asarray(Wq, dtype=np.float32)).astype(np_dt)
    wk = np.ascontiguousarray(np.asarray(Wk, dtype=np.float32)).astype(np_dt)
    wv = np.ascontiguousarray(np.asarray(Wv, dtype=np.float32)).astype(np_dt)

    in_maps = []
    for b in range(BS):
        in_maps.append(
            {
                "xT": np.ascontiguousarray(x[b].T).astype(np_dt),
                "Wq": wq,
                "Wk": wk,
                "Wv": wv,
            }
        )

    res = run_bass_kernel_spmd(nc, in_maps, core_ids=list(range(BS)))
    LAST_RESULT = res
    out = np.stack(
        [np.asarray(res.results[b]["out"], dtype=np.float32) for b in range(BS)],
        axis=0,
    )
    return out


# revision 4
# speedup vs baseline: 1.5863x; 1.5863x over previous
"""Causal single-head attention on 8 TRN2 NeuronCores.

Data-parallel over batch: core b computes attention for batch element b.
Inputs are pre-transposed on the host (xT = x[b].T) so the device kernel
never has to transpose the activations; causality is exploited on-device
(only score tiles on/below the diagonal are computed; the diagonal tile is
masked with affine_select). The softmax denominator comes for free from a
ones-column appended to V in the PV matmul.
"""

import os
import sys

sys.path.insert(0, "/opt/trn_rl_repo")

import numpy as np

BS, SEQ, D, E = 8, 2048, 768, 64
P = 128                 # SBUF partitions
CHUNK = 512             # q-chunk (matmul moving free dim)
N_CHUNKS = SEQ // CHUNK  # 4
N_KT = SEQ // P          # 16 key tiles
N_DT = D // P            # 6 contraction tiles for the projections
SCALE = 1.0 / np.sqrt(E).astype(np.float32)  # 0.125

# Compute dtype for the matmul operands ("float32", "bfloat16", "float32r")
DT_MM_NAME = os.environ.get("ATTN_DT_MM", "float32")

_CACHE = {}

LAST_RESULT = None  # BassKernelResults of the most recent run (for test.py)


def _build(dt_mm_name):
    from contextlib import ExitStack

    import concourse.bass as bass  # noqa: F401  (import registers engines)
    import concourse.tile as tile
    from concourse import bacc, mybir
    from concourse.masks import make_identity

    f32 = mybir.dt.float32
    dt_mm = getattr(mybir.dt, dt_mm_name)
    # DRAM layout dtype: float32r is a matmul-operand view; store as f32.
    dt_in = f32 if dt_mm_name == "float32r" else dt_mm

    nc = bacc.Bacc(
        "TRN2", target_bir_lowering=False, debug=False, num_devices=BS
    )
    xT_d = nc.dram_tensor("xT", [D, SEQ], dt_in, kind="ExternalInput").ap()
    wq_d = nc.dram_tensor("Wq", [D, E], dt_in, kind="ExternalInput").ap()
    wk_d = nc.dram_tensor("Wk", [D, E], dt_in, kind="ExternalInput").ap()
    wv_d = nc.dram_tensor("Wv", [D, E], dt_in, kind="ExternalInput").ap()
    out_d = nc.dram_tensor("out", [SEQ, E], f32, kind="ExternalOutput").ap()

    def as_mm(ap):
        # reinterpret an f32 SBUF AP as float32r for the matmul operands
        if dt_mm_name == "float32r":
            return ap.bitcast(mybir.dt.float32r)
        return ap

    with tile.TileContext(nc) as tc, ExitStack() as ctx:
        const = ctx.enter_context(tc.tile_pool(name="const", bufs=1))
        mm_ps = ctx.enter_context(tc.tile_pool(name="mm_ps", bufs=3, space="PSUM"))
        pv_ps = ctx.enter_context(tc.tile_pool(name="pv_ps", bufs=2, space="PSUM"))
        ot_ps = ctx.enter_context(tc.tile_pool(name="ot_ps", bufs=2, space="PSUM"))
        p_pool = ctx.enter_context(tc.tile_pool(name="p_pool", bufs=4))
        o_pool = ctx.enter_context(tc.tile_pool(name="o_pool", bufs=2))

        ident = const.tile([P, P], f32)
        make_identity(nc, ident)
        if dt_in != f32:
            ident_mm = const.tile([P, P], dt_in)
            make_identity(nc, ident_mm)
        else:
            ident_mm = ident

        # --- load weights and xT ---
        w_sbs = []
        for name, w_d in (("wq", wq_d), ("wk", wk_d), ("wv", wv_d)):
            w_sb = const.tile([P, N_DT, E], dt_in, tag=name)
            for d in range(N_DT):
                nc.sync.dma_start(w_sb[:, d, :], w_d[d * P:(d + 1) * P, :])
            w_sbs.append(w_sb)
        wq_sb, wk_sb, wv_sb = w_sbs

        xT_sb = const.tile([P, N_DT, SEQ], dt_in, tag="xT")
        for d in range(N_DT):
            nc.sync.dma_start(xT_sb[:, d, :], xT_d[d * P:(d + 1) * P, :])

        # --- projections: qT/kT/vT = W.T @ xT, [64, SEQ] each ---
        qT_sb = const.tile([E, SEQ], dt_in, tag="qT")
        kT_sb = const.tile([E, SEQ], dt_in, tag="kT")
        vT_sb = const.tile([E, SEQ], dt_in, tag="vT")
        for c in range(N_CHUNKS):
            sl = slice(c * CHUNK, (c + 1) * CHUNK)
            for w_sb, dst in ((wq_sb, qT_sb), (wk_sb, kT_sb), (wv_sb, vT_sb)):
                ps = mm_ps.tile([E, CHUNK], f32, tag="mm")
                for d in range(N_DT):
                    nc.tensor.matmul(
                        ps,
                        lhsT=as_mm(w_sb[:, d, :]),
                        rhs=as_mm(xT_sb[:, d, sl]),
                        start=(d == 0),
                        stop=(d == N_DT - 1),
                    )
                nc.vector.tensor_copy(dst[:, sl], ps)

        # --- v natural [128, 16, 65]: transpose vT back, ones in col E ---
        v_sb = const.tile([P, N_KT, E + 1], dt_in, tag="v")
        nc.vector.memset(v_sb[:], 1.0)
        for t in range(N_KT):
            vt = ot_ps.tile([P, E], dt_in, tag="ot")
            nc.tensor.transpose(
                vt, vT_sb[:, t * P:(t + 1) * P], ident_mm[0:E, 0:E]
            )
            nc.vector.tensor_copy(v_sb[:, t, 0:E], vt)

        # --- flash loop over q-chunks ---
        for c in range(N_CHUNKS):
            sl = slice(c * CHUNK, (c + 1) * CHUNK)
            n_sk = 4 * (c + 1)  # causal: only key tiles covering k <= q
            pv = pv_ps.tile([E + 1, CHUNK], f32, tag="pv")
            for t in range(n_sk):
                s_ps = mm_ps.tile([P, CHUNK], f32, tag="mm")
                nc.tensor.matmul(
                    s_ps,
                    lhsT=as_mm(kT_sb[:, t * P:(t + 1) * P]),
                    rhs=as_mm(qT_sb[:, sl]),
                    start=True,
                    stop=True,
                )
                p_sb = p_pool.tile([P, CHUNK], dt_in, tag="p")
                nc.scalar.activation(
                    p_sb, s_ps, mybir.ActivationFunctionType.Exp, scale=float(SCALE)
                )
                if t >= 4 * c:
                    # diagonal tile: zero out entries with k > q
                    j = t - 4 * c
                    nc.gpsimd.affine_select(
                        out=p_sb,
                        in_=p_sb,
                        compare_op=mybir.AluOpType.is_ge,
                        fill=0.0,
                        base=-j * P,
                        pattern=[[1, CHUNK]],
                        channel_multiplier=-1,
                    )
                nc.tensor.matmul(
                    pv,
                    lhsT=as_mm(v_sb[:, t, :]),
                    rhs=as_mm(p_sb[:]),
                    start=(t == 0),
                    stop=(t == n_sk - 1),
                )
            # transpose [E+1, CHUNK] -> 4x [128, E+1], normalize, store
            pvT_sb = o_pool.tile([E + 1, CHUNK], f32, tag="pvT")
            nc.vector.tensor_copy(pvT_sb, pv)
            for st in range(CHUNK // P):
                ot = ot_ps.tile([P, E + 1], f32, tag="ot")
                nc.tensor.transpose(
                    ot,
                    pvT_sb[:, st * P:(st + 1) * P],
                    ident[0:E + 1, 0:E + 1],
                )
                recip = o_pool.tile([P, 1], f32, tag="recip")
                nc.vector.reciprocal(recip, ot[:, E:E + 1])
                o_sb = o_pool.tile([P, E], f32, tag="o")
                nc.vector.tensor_scalar_mul(o_sb, ot[:, 0:E], recip)
                r0 = c * CHUNK + st * P
                nc.sync.dma_start(out_d[r0:r0 + P, :], o_sb)

    nc.compile()
    return nc


def _get(dt_mm_name=None):
    name = dt_mm_name or DT_MM_NAME
    if name not in _CACHE:
        _CACHE[name] = _build(name)
    return _CACHE[name]


def _ensure_axon_hooks():
    """The agent image's antenv lacks axon_hooks; bass_utils imports it when
    trace=True under axon. Provide it, wired to the real ctypes NTFF
    profiler from trn_agent_boot when available."""
    try:
        import antenv.axon_hooks  # noqa: F401

        return
    except ImportError:
        pass
    import types

    try:
        import antenv
    except ImportError:
        return
    mod = types.ModuleType("antenv.axon_hooks")
    mod._hook = None

    def set_axon_ntff_profile_hook(h):
        mod._hook = h

    def get_axon_ntff_profile_hook():
        return mod._hook

    mod.set_axon_ntff_profile_hook = set_axon_ntff_profile_hook
    mod.get_axon_ntff_profile_hook = get_axon_ntff_profile_hook
    sys.modules["antenv.axon_hooks"] = mod
    antenv.axon_hooks = mod
    try:
        from trn_agent_boot.trn_boot import _ntff_profile_via_ctypes

        so_path = "/opt/axon/libaxon_pjrt.so"
        if os.path.exists(so_path):
            mod._hook = _ntff_profile_via_ctypes(so_path)
    except Exception:
        pass


def kernel(x, mask, Wq, Wk, Wv):
    global LAST_RESULT
    _ensure_axon_hooks()
    from concourse.bass_utils import run_bass_kernel_spmd

    nc = _get()

    if DT_MM_NAME == "bfloat16":
        import ml_dtypes

        np_dt = ml_dtypes.bfloat16
    else:
        np_dt = np.float32

    x = np.asarray(x, dtype=np.float32)
    wq = np.ascontiguousarray(np.asarray(Wq, dtype=np.float32)).astype(np_dt)
    wk = np.ascontiguousarray(np.asarray(Wk, dtype=np.float32)).astype(np_dt)
    wv = np.ascontiguousarray(np.asarray(Wv, dtype=np.float32)).astype(np_dt)

    in_maps = []
    for b in range(BS):
        in_maps.append(
            {
                "xT": np.ascontiguousarray(x[b].T).astype(np_dt),
                "Wq": wq,
                "Wk": wk,
                "Wv": wv,
            }
        )

    res = run_bass_kernel_spmd(nc, in_maps, core_ids=list(range(BS)))
    LAST_RESULT = res
    out = np.stack(
        [np.asarray(res.results[b]["out"], dtype=np.float32) for b in range(BS)],
        axis=0,
    )
    return out


# revision 6
# speedup vs baseline: 2.0718x; 1.3061x over previous
"""Causal single-head attention on 8 TRN2 NeuronCores.

Data-parallel over batch: core b computes attention for batch element b.
Inputs are pre-transposed on the host (xT = x[b].T) so the device kernel
never transposes activations. Causality is exploited on-device: only
score tiles on/below the diagonal are computed; diagonal tiles are
masked by multiplying with precomputed 0/1 tiles. The softmax
denominator comes for free from a ones-column appended to V in the PV
matmul.

Layout tricks:
- [Wq|Wk] packed into one 128-col stationary: the projection matmul
  produces qT on partitions 0-63 and kT on partitions 64-127 (tensor A);
  a partition-swapped mirror (tensor B) is made with SBUF->SBUF DMAs.
- Score matmuls contract over e=64, i.e. half the PE array. Pairs of
  key tiles run concurrently in array row-groups (0,0) and (64,0), fed
  from A/B at the matching base partitions.
- exp runs once per pair ([128, 2x512] PSUM span) on the Scalar engine.
"""

import os
import sys

sys.path.insert(0, "/opt/trn_rl_repo")

import numpy as np

BS, SEQ, D, E = 8, 2048, 768, 64
P = 128                  # SBUF partitions
CHUNK = 512              # q-chunk (matmul moving free dim)
N_CHUNKS = SEQ // CHUNK  # 4
N_KT = SEQ // P          # 16 key tiles
N_DT = D // P            # 6 contraction tiles for the projections
SCALE = 1.0 / np.sqrt(E).astype(np.float32)  # 0.125

DT_MM_NAME = os.environ.get("ATTN_DT_MM", "bfloat16")

_CACHE = {}

LAST_RESULT = None  # BassKernelResults of the most recent run (for test.py)


def _build(dt_mm_name):
    from contextlib import ExitStack

    import concourse.bass as bass  # noqa: F401
    import concourse.tile as tile
    from concourse import bacc, mybir
    from concourse.masks import make_identity

    f32 = mybir.dt.float32
    dt_in = getattr(mybir.dt, dt_mm_name)

    nc = bacc.Bacc(
        "TRN2", target_bir_lowering=False, debug=False, num_devices=BS
    )
    xT_d = nc.dram_tensor("xT", [D, SEQ], dt_in, kind="ExternalInput").ap()
    wqk_d = nc.dram_tensor("Wqk", [D, 2 * E], dt_in, kind="ExternalInput").ap()
    wv_d = nc.dram_tensor("Wv", [D, E], dt_in, kind="ExternalInput").ap()
    out_d = nc.dram_tensor("out", [SEQ, E], f32, kind="ExternalOutput").ap()

    with tile.TileContext(nc) as tc, ExitStack() as ctx:
        const = ctx.enter_context(tc.tile_pool(name="const", bufs=1))
        mm_ps = ctx.enter_context(tc.tile_pool(name="mm_ps", bufs=2, space="PSUM"))
        pv_ps = ctx.enter_context(tc.tile_pool(name="pv_ps", bufs=2, space="PSUM"))
        ot_ps = ctx.enter_context(tc.tile_pool(name="ot_ps", bufs=2, space="PSUM"))
        p_pool = ctx.enter_context(tc.tile_pool(name="p_pool", bufs=4))
        o_pool = ctx.enter_context(tc.tile_pool(name="o_pool", bufs=2))

        ident = const.tile([P, P], f32)
        make_identity(nc, ident)
        if dt_in != f32:
            ident_mm = const.tile([P, P], dt_in)
            make_identity(nc, ident_mm)
        else:
            ident_mm = ident

        # diagonal-pair causal masks (0/1), one per pair offset j0 in {0, 2}
        # elem[x, i1, y] = 1.0 iff y >= x + 128*(j0+i1)
        dmasks = []
        for j0 in (0, 2):
            m = const.tile([P, 2, CHUNK], dt_in, tag=f"dmask{j0}")
            nc.gpsimd.memset(m[:], 1.0)
            nc.gpsimd.affine_select(
                out=m[:],
                in_=m[:],
                compare_op=mybir.AluOpType.is_ge,
                fill=0.0,
                base=-j0 * P,
                pattern=[[-P, 2], [1, CHUNK]],
                channel_multiplier=-1,
            )
            dmasks.append(m)

        # --- weights ---
        wqk_sb = const.tile([P, N_DT, 2 * E], dt_in, tag="wqk")
        for d in range(N_DT):
            nc.sync.dma_start(wqk_sb[:, d, :], wqk_d[d * P:(d + 1) * P, :])
        wv_sb = const.tile([P, N_DT, E], dt_in, tag="wv")
        for d in range(N_DT):
            nc.sync.dma_start(wv_sb[:, d, :], wv_d[d * P:(d + 1) * P, :])

        # --- xT, DMA'd chunk-major so the first projection starts early ---
        xT_sb = const.tile([P, N_DT, SEQ], dt_in, tag="xT")
        for c in range(N_CHUNKS):
            sl = slice(c * CHUNK, (c + 1) * CHUNK)
            for d in range(N_DT):
                nc.sync.dma_start(xT_sb[:, d, sl], xT_d[d * P:(d + 1) * P, sl])

        # --- projections ---
        # A: partitions 0-63 = qT, 64-127 = kT;  B: the partition-swap of A
        A_sb = const.tile([P, SEQ], dt_in, tag="A")
        B_sb = const.tile([P, SEQ], dt_in, tag="B")
        vT_sb = const.tile([E, SEQ], dt_in, tag="vT")
        for c in range(N_CHUNKS):
            sl = slice(c * CHUNK, (c + 1) * CHUNK)
            ps = mm_ps.tile([P, 2, CHUNK], f32, tag="mm")
            for d in range(N_DT):
                nc.tensor.matmul(
                    ps[:, 0, :],
                    lhsT=wqk_sb[:, d, :],
                    rhs=xT_sb[:, d, sl],
                    start=(d == 0),
                    stop=(d == N_DT - 1),
                )
            nc.vector.tensor_copy(A_sb[:, sl], ps[:, 0, :])
            nc.sync.dma_start(B_sb[0:E, sl], A_sb[E:P, sl])
            nc.sync.dma_start(B_sb[E:P, sl], A_sb[0:E, sl])

            psv = mm_ps.tile([E, 2, CHUNK], f32, tag="mm")
            for d in range(N_DT):
                nc.tensor.matmul(
                    psv[:, 0, :],
                    lhsT=wv_sb[:, d, :],
                    rhs=xT_sb[:, d, sl],
                    start=(d == 0),
                    stop=(d == N_DT - 1),
                )
            nc.vector.tensor_copy(vT_sb[:, sl], psv[:, 0, :])

        # --- v natural [128, 16, 65]: transpose vT back, ones in col E ---
        v_sb = const.tile([P, N_KT, E + 1], dt_in, tag="v")
        nc.vector.memset(v_sb[:], 1.0)
        for t in range(N_KT):
            vt = ot_ps.tile([P, E], dt_in, tag="ot")
            nc.tensor.transpose(
                vt, vT_sb[:, t * P:(t + 1) * P], ident_mm[0:E, 0:E]
            )
            nc.vector.tensor_copy(v_sb[:, t, 0:E], vt)

        # --- flash loop over q-chunks ---
        for c in range(N_CHUNKS):
            sl = slice(c * CHUNK, (c + 1) * CHUNK)
            n_pairs = 2 * (c + 1)  # causal: key tiles 0 .. 4c+3 in pairs
            pv = pv_ps.tile([E + 1, CHUNK], f32, tag="pv")
            for pi in range(n_pairs):
                t0, t1 = 2 * pi, 2 * pi + 1
                s2 = mm_ps.tile([P, 2, CHUNK], f32, tag="mm")
                # row-group (0,0): kT/qT from base partition 0
                nc.tensor.matmul(
                    s2[:, 0, :],
                    lhsT=B_sb[0:E, t0 * P:(t0 + 1) * P],
                    rhs=A_sb[0:E, sl],
                    start=True,
                    stop=True,
                )
                # row-group (64,0): kT/qT from base partition 64
                nc.tensor.matmul(
                    s2[:, 1, :],
                    lhsT=A_sb[E:P, t1 * P:(t1 + 1) * P],
                    rhs=B_sb[E:P, sl],
                    start=True,
                    stop=True,
                )
                p2 = p_pool.tile([P, 2, CHUNK], dt_in, tag="p")
                nc.scalar.activation(
                    p2[:], s2[:], mybir.ActivationFunctionType.Exp,
                    scale=float(SCALE),
                )
                if t0 >= 4 * c:  # diagonal pair: zero entries with k > q
                    nc.vector.tensor_mul(
                        p2[:], p2[:], dmasks[(t0 - 4 * c) // 2][:]
                    )
                nc.tensor.matmul(
                    pv,
                    lhsT=v_sb[:, t0, :],
                    rhs=p2[:, 0, :],
                    start=(pi == 0),
                    stop=False,
                )
                nc.tensor.matmul(
                    pv,
                    lhsT=v_sb[:, t1, :],
                    rhs=p2[:, 1, :],
                    start=False,
                    stop=(pi == n_pairs - 1),
                )
            # transpose [E+1, CHUNK] -> 4x [128, E+1], normalize, store
            pvT_sb = o_pool.tile([E + 1, CHUNK], f32, tag="pvT")
            nc.vector.tensor_copy(pvT_sb, pv)
            for st in range(CHUNK // P):
                ot = ot_ps.tile([P, E + 1], f32, tag="ot")
                nc.tensor.transpose(
                    ot,
                    pvT_sb[:, st * P:(st + 1) * P],
                    ident[0:E + 1, 0:E + 1],
                )
                recip = o_pool.tile([P, 1], f32, tag="recip")
                nc.vector.reciprocal(recip, ot[:, E:E + 1])
                o_sb = o_pool.tile([P, E], f32, tag="o")
                nc.vector.tensor_scalar_mul(o_sb, ot[:, 0:E], recip)
                r0 = c * CHUNK + st * P
                nc.sync.dma_start(out_d[r0:r0 + P, :], o_sb)

    nc.compile()
    return nc


def _get(dt_mm_name=None):
    name = dt_mm_name or DT_MM_NAME
    if name not in _CACHE:
        _CACHE[name] = _build(name)
    return _CACHE[name]


def _ensure_axon_hooks():
    """The agent image's antenv lacks axon_hooks; bass_utils imports it when
    trace=True under axon. Provide it, wired to the real ctypes NTFF
    profiler from trn_agent_boot when available."""
    try:
        import antenv.axon_hooks  # noqa: F401

        return
    except ImportError:
        pass
    import types

    try:
        import antenv
    except ImportError:
        return
    mod = types.ModuleType("antenv.axon_hooks")
    mod._hook = None

    def set_axon_ntff_profile_hook(h):
        mod._hook = h

    def get_axon_ntff_profile_hook():
        return mod._hook

    mod.set_axon_ntff_profile_hook = set_axon_ntff_profile_hook
    mod.get_axon_ntff_profile_hook = get_axon_ntff_profile_hook
    sys.modules["antenv.axon_hooks"] = mod
    antenv.axon_hooks = mod
    try:
        from trn_agent_boot.trn_boot import _ntff_profile_via_ctypes

        so_path = "/opt/axon/libaxon_pjrt.so"
        if os.path.exists(so_path):
            mod._hook = _ntff_profile_via_ctypes(so_path)
    except Exception:
        pass


def kernel(x, mask, Wq, Wk, Wv):
    global LAST_RESULT
    _ensure_axon_hooks()
    from concourse.bass_utils import run_bass_kernel_spmd

    nc = _get()

    if DT_MM_NAME == "bfloat16":
        import ml_dtypes

        np_dt = ml_dtypes.bfloat16
    else:
        np_dt = np.float32

    x = np.asarray(x, dtype=np.float32)
    wqk = np.concatenate(
        [np.asarray(Wq, dtype=np.float32), np.asarray(Wk, dtype=np.float32)],
        axis=1,
    ).astype(np_dt)
    wv = np.ascontiguousarray(np.asarray(Wv, dtype=np.float32)).astype(np_dt)

    in_maps = []
    for b in range(BS):
        in_maps.append(
            {
                "xT": np.ascontiguousarray(x[b].T).astype(np_dt),
                "Wqk": wqk,
                "Wv": wv,
            }
        )

    res = run_bass_kernel_spmd(nc, in_maps, core_ids=list(range(BS)))
    LAST_RESULT = res
    out = np.stack(
        [np.asarray(res.results[b]["out"], dtype=np.float32) for b in range(BS)],
        axis=0,
    )
    return out


# revision 7
# speedup vs baseline: 2.5776x; 1.2442x over previous
"""Causal single-head attention on 8 TRN2 NeuronCores.

Data-parallel over batch: core b computes attention for batch element b.
Inputs are pre-transposed on the host (xT = x[b].T) so the device kernel
never transposes activations. Causality is exploited on-device: only
score tiles on/below the diagonal are computed; diagonal tiles are
masked by multiplying with precomputed 0/1 tiles. The softmax
denominator comes for free from a ones-column appended to V in the PV
matmul.

Layout tricks:
- [Wq|Wk] packed into one 128-col stationary: the projection matmul
  produces qT on partitions 0-63 and kT on partitions 64-127 (tensor A);
  a partition-swapped mirror (tensor B) is made with SBUF->SBUF DMAs.
- Score matmuls contract over e=64, i.e. half the PE array. Pairs of
  key tiles run concurrently in array row-groups (0,0) and (64,0), fed
  from A/B at the matching base partitions.
- exp runs once per pair ([128, 2x512] PSUM span) on the Scalar engine.
"""

import os
import sys

sys.path.insert(0, "/opt/trn_rl_repo")

import numpy as np

BS, SEQ, D, E = 8, 2048, 768, 64
P = 128                  # SBUF partitions
CHUNK = 512              # q-chunk (matmul moving free dim)
N_CHUNKS = SEQ // CHUNK  # 4
N_KT = SEQ // P          # 16 key tiles
N_DT = D // P            # 6 contraction tiles for the projections
SCALE = 1.0 / np.sqrt(E).astype(np.float32)  # 0.125

DT_MM_NAME = os.environ.get("ATTN_DT_MM", "bfloat16")

_CACHE = {}

LAST_RESULT = None  # BassKernelResults of the most recent run (for test.py)


def _build(dt_mm_name):
    from contextlib import ExitStack

    import concourse.bass as bass  # noqa: F401
    import concourse.tile as tile
    from concourse import bacc, mybir
    from concourse.masks import make_identity

    f32 = mybir.dt.float32
    dt_in = getattr(mybir.dt, dt_mm_name)

    nc = bacc.Bacc(
        "TRN2", target_bir_lowering=False, debug=False, num_devices=BS
    )
    xT_d = nc.dram_tensor("xT", [D, SEQ], dt_in, kind="ExternalInput").ap()
    wqk_d = nc.dram_tensor("Wqk", [D, 2 * E], dt_in, kind="ExternalInput").ap()
    wv_d = nc.dram_tensor("Wv", [D, E], dt_in, kind="ExternalInput").ap()
    out_d = nc.dram_tensor("out", [SEQ, E], f32, kind="ExternalOutput").ap()

    with tile.TileContext(nc) as tc, ExitStack() as ctx:
        const = ctx.enter_context(tc.tile_pool(name="const", bufs=1))
        mm_ps = ctx.enter_context(tc.tile_pool(name="mm_ps", bufs=2, space="PSUM"))
        pv_ps = ctx.enter_context(tc.tile_pool(name="pv_ps", bufs=2, space="PSUM"))
        ot_ps = ctx.enter_context(tc.tile_pool(name="ot_ps", bufs=2, space="PSUM"))
        p_pool = ctx.enter_context(tc.tile_pool(name="p_pool", bufs=4))
        o_pool = ctx.enter_context(tc.tile_pool(name="o_pool", bufs=2))

        ident = const.tile([P, P], f32)
        make_identity(nc, ident)
        if dt_in != f32:
            ident_mm = const.tile([P, P], dt_in)
            make_identity(nc, ident_mm)
        else:
            ident_mm = ident

        # diagonal-pair causal masks (0/1), one per pair offset j0 in {0, 2}
        # elem[x, i1, y] = 1.0 iff y >= x + 128*(j0+i1)
        dmasks = []
        for j0 in (0, 2):
            m = const.tile([P, 2, CHUNK], dt_in, tag=f"dmask{j0}")
            nc.gpsimd.memset(m[:], 1.0)
            nc.gpsimd.affine_select(
                out=m[:],
                in_=m[:],
                compare_op=mybir.AluOpType.is_ge,
                fill=0.0,
                base=-j0 * P,
                pattern=[[-P, 2], [1, CHUNK]],
                channel_multiplier=-1,
            )
            dmasks.append(m)

        # --- weights: one rearranged DMA each, on the scalar HWDGE ring ---
        wqk_sb = const.tile([P, N_DT, 2 * E], dt_in, tag="wqk")
        nc.scalar.dma_start(
            wqk_sb[:], wqk_d.rearrange("(dt p) e -> p dt e", p=P)
        )
        wv_sb = const.tile([P, N_DT, E], dt_in, tag="wv")
        nc.scalar.dma_start(wv_sb[:], wv_d.rearrange("(dt p) e -> p dt e", p=P))

        # --- xT: one DMA per chunk on the sync ring (projection c0 can
        # start after the first ~0.8MB instead of the full 3MB) ---
        xT_sb = const.tile([P, N_DT, SEQ], dt_in, tag="xT")
        xT_re = xT_d.rearrange("(dt p) s -> p dt s", p=P)
        for c in range(N_CHUNKS):
            sl = slice(c * CHUNK, (c + 1) * CHUNK)
            nc.sync.dma_start(xT_sb[:, :, sl], xT_re[:, :, sl])

        # --- projections ---
        # A: partitions 0-63 = qT, 64-127 = kT;  B: the partition-swap of A
        A_sb = const.tile([P, SEQ], dt_in, tag="A")
        B_sb = const.tile([P, SEQ], dt_in, tag="B")
        vT_sb = const.tile([E, SEQ], dt_in, tag="vT")
        for c in range(N_CHUNKS):
            sl = slice(c * CHUNK, (c + 1) * CHUNK)
            ps = mm_ps.tile([P, 2, CHUNK], f32, tag="mm")
            for d in range(N_DT):
                nc.tensor.matmul(
                    ps[:, 0, :],
                    lhsT=wqk_sb[:, d, :],
                    rhs=xT_sb[:, d, sl],
                    start=(d == 0),
                    stop=(d == N_DT - 1),
                )
            nc.scalar.copy(A_sb[:, sl], ps[:, 0, :])
            # partition-shifted mirrors (validated on HW): kT -> base 0,
            # qT -> base 64
            nc.vector.tensor_copy(B_sb[0:E, sl], A_sb[E:P, sl])
            nc.vector.tensor_copy(B_sb[E:P, sl], A_sb[0:E, sl])

            psv = mm_ps.tile([E, 2, CHUNK], f32, tag="mm")
            for d in range(N_DT):
                nc.tensor.matmul(
                    psv[:, 0, :],
                    lhsT=wv_sb[:, d, :],
                    rhs=xT_sb[:, d, sl],
                    start=(d == 0),
                    stop=(d == N_DT - 1),
                )
            nc.vector.tensor_copy(vT_sb[:, sl], psv[:, 0, :])

        # --- v natural [128, 16, 65]: transpose vT back, ones in col E ---
        v_sb = const.tile([P, N_KT, E + 1], dt_in, tag="v")
        nc.vector.memset(v_sb[:], 1.0)
        for t in range(N_KT):
            vt = ot_ps.tile([P, E], dt_in, tag="ot")
            nc.tensor.transpose(
                vt, vT_sb[:, t * P:(t + 1) * P], ident_mm[0:E, 0:E]
            )
            nc.vector.tensor_copy(v_sb[:, t, 0:E], vt)

        # --- flash loop over q-chunks ---
        for c in range(N_CHUNKS):
            sl = slice(c * CHUNK, (c + 1) * CHUNK)
            n_pairs = 2 * (c + 1)  # causal: key tiles 0 .. 4c+3 in pairs
            pv = pv_ps.tile([E + 1, CHUNK], f32, tag="pv")
            for pi in range(n_pairs):
                t0, t1 = 2 * pi, 2 * pi + 1
                s2 = mm_ps.tile([P, 2, CHUNK], f32, tag="mm")
                # row-group (0,0): kT/qT from base partition 0
                nc.tensor.matmul(
                    s2[:, 0, :],
                    lhsT=B_sb[0:E, t0 * P:(t0 + 1) * P],
                    rhs=A_sb[0:E, sl],
                    start=True,
                    stop=True,
                )
                # row-group (64,0): kT/qT from base partition 64
                nc.tensor.matmul(
                    s2[:, 1, :],
                    lhsT=A_sb[E:P, t1 * P:(t1 + 1) * P],
                    rhs=B_sb[E:P, sl],
                    start=True,
                    stop=True,
                )
                p2 = p_pool.tile([P, 2, CHUNK], dt_in, tag="p")
                nc.scalar.activation(
                    p2[:], s2[:], mybir.ActivationFunctionType.Exp,
                    scale=float(SCALE),
                )
                if t0 >= 4 * c:  # diagonal pair: zero entries with k > q
                    nc.vector.tensor_mul(
                        p2[:], p2[:], dmasks[(t0 - 4 * c) // 2][:]
                    )
                nc.tensor.matmul(
                    pv,
                    lhsT=v_sb[:, t0, :],
                    rhs=p2[:, 0, :],
                    start=(pi == 0),
                    stop=False,
                )
                nc.tensor.matmul(
                    pv,
                    lhsT=v_sb[:, t1, :],
                    rhs=p2[:, 1, :],
                    start=False,
                    stop=(pi == n_pairs - 1),
                )
            # transpose [E+1, CHUNK] -> 4x [128, E+1], normalize, store
            pvT_sb = o_pool.tile([E + 1, CHUNK], f32, tag="pvT")
            nc.vector.tensor_copy(pvT_sb, pv)
            for st in range(CHUNK // P):
                ot = ot_ps.tile([P, E + 1], f32, tag="ot")
                nc.tensor.transpose(
                    ot,
                    pvT_sb[:, st * P:(st + 1) * P],
                    ident[0:E + 1, 0:E + 1],
                )
                recip = o_pool.tile([P, 1], f32, tag="recip")
                nc.vector.reciprocal(recip, ot[:, E:E + 1])
                o_sb = o_pool.tile([P, E], f32, tag="o")
                nc.vector.tensor_scalar_mul(o_sb, ot[:, 0:E], recip)
                r0 = c * CHUNK + st * P
                nc.sync.dma_start(out_d[r0:r0 + P, :], o_sb)

    nc.compile()
    return nc


def _get(dt_mm_name=None):
    name = dt_mm_name or DT_MM_NAME
    if name not in _CACHE:
        _CACHE[name] = _build(name)
    return _CACHE[name]


def _ensure_axon_hooks():
    """The agent image's antenv lacks axon_hooks; bass_utils imports it when
    trace=True under axon. Provide it, wired to the real ctypes NTFF
    profiler from trn_agent_boot when available."""
    try:
        import antenv.axon_hooks  # noqa: F401

        return
    except ImportError:
        pass
    import types

    try:
        import antenv
    except ImportError:
        return
    mod = types.ModuleType("antenv.axon_hooks")
    mod._hook = None

    def set_axon_ntff_profile_hook(h):
        mod._hook = h

    def get_axon_ntff_profile_hook():
        return mod._hook

    mod.set_axon_ntff_profile_hook = set_axon_ntff_profile_hook
    mod.get_axon_ntff_profile_hook = get_axon_ntff_profile_hook
    sys.modules["antenv.axon_hooks"] = mod
    antenv.axon_hooks = mod
    try:
        from trn_agent_boot.trn_boot import _ntff_profile_via_ctypes

        so_path = "/opt/axon/libaxon_pjrt.so"
        if os.path.exists(so_path):
            mod._hook = _ntff_profile_via_ctypes(so_path)
    except Exception:
        pass


def kernel(x, mask, Wq, Wk, Wv):
    global LAST_RESULT
    _ensure_axon_hooks()
    from concourse.bass_utils import run_bass_kernel_spmd

    nc = _get()

    if DT_MM_NAME == "bfloat16":
        import ml_dtypes

        np_dt = ml_dtypes.bfloat16
    else:
        np_dt = np.float32

    x = np.asarray(x, dtype=np.float32)
    wqk = np.concatenate(
        [np.asarray(Wq, dtype=np.float32), np.asarray(Wk, dtype=np.float32)],
        axis=1,
    ).astype(np_dt)
    wv = np.ascontiguousarray(np.asarray(Wv, dtype=np.float32)).astype(np_dt)

    in_maps = []
    for b in range(BS):
        in_maps.append(
            {
                "xT": np.ascontiguousarray(x[b].T).astype(np_dt),
                "Wqk": wqk,
                "Wv": wv,
            }
        )

    res = run_bass_kernel_spmd(nc, in_maps, core_ids=list(range(BS)))
    LAST_RESULT = res
    out = np.stack(
        [np.asarray(res.results[b]["out"], dtype=np.float32) for b in range(BS)],
        axis=0,
    )
    return out


# revision 10
# speedup vs baseline: 2.7528x; 1.0680x over previous
"""Causal single-head attention on 8 TRN2 NeuronCores.

Data-parallel over batch: core b computes attention for batch element b.
Inputs are pre-transposed on the host (xT = x[b].T) so the device kernel
never transposes activations. Causality is exploited on-device: only
score tiles on/below the diagonal are computed; diagonal tiles are
masked by multiplying with precomputed 0/1 tiles. The softmax
denominator comes for free from a ones-column appended to V in the PV
matmul.

Layout tricks:
- [Wq|Wk] packed into one 128-col stationary: the projection matmul
  produces qT on partitions 0-63 and kT on partitions 64-127 (tensor A);
  a partition-swapped mirror (tensor B) is made with SBUF->SBUF DMAs.
- Score matmuls contract over e=64, i.e. half the PE array. Pairs of
  key tiles run concurrently in array row-groups (0,0) and (64,0), fed
  from A/B at the matching base partitions.
- exp runs once per pair ([128, 2x512] PSUM span) on the Scalar engine.
"""

import os
import sys

sys.path.insert(0, "/opt/trn_rl_repo")

import numpy as np

BS, SEQ, D, E = 8, 2048, 768, 64
P = 128                  # SBUF partitions
CHUNK = 512              # q-chunk (matmul moving free dim)
N_CHUNKS = SEQ // CHUNK  # 4
N_KT = SEQ // P          # 16 key tiles
N_DT = D // P            # 6 contraction tiles for the projections
SCALE = 1.0 / np.sqrt(E).astype(np.float32)  # 0.125

DT_MM_NAME = os.environ.get("ATTN_DT_MM", "bfloat16")

_CACHE = {}

LAST_RESULT = None  # BassKernelResults of the most recent run (for test.py)


def _build(dt_mm_name):
    from contextlib import ExitStack

    import concourse.bass as bass  # noqa: F401
    import concourse.tile as tile
    from concourse import bacc, mybir
    from concourse.masks import make_identity

    f32 = mybir.dt.float32
    dt_in = getattr(mybir.dt, dt_mm_name)

    nc = bacc.Bacc(
        "TRN2", target_bir_lowering=False, debug=False, num_devices=BS
    )
    xT_d = nc.dram_tensor("xT", [D, SEQ], dt_in, kind="ExternalInput").ap()
    wqk_d = nc.dram_tensor("Wqk", [D, 2 * E], dt_in, kind="ExternalInput").ap()
    wv_d = nc.dram_tensor("Wv", [D, E], dt_in, kind="ExternalInput").ap()
    out_d = nc.dram_tensor("out", [SEQ, E], f32, kind="ExternalOutput").ap()

    with tile.TileContext(nc) as tc, ExitStack() as ctx:
        const = ctx.enter_context(tc.tile_pool(name="const", bufs=1))
        mm_ps = ctx.enter_context(tc.tile_pool(name="mm_ps", bufs=2, space="PSUM"))
        pv_ps = ctx.enter_context(tc.tile_pool(name="pv_ps", bufs=2, space="PSUM"))
        ot_ps = ctx.enter_context(tc.tile_pool(name="ot_ps", bufs=2, space="PSUM"))
        p_pool = ctx.enter_context(tc.tile_pool(name="p_pool", bufs=4))
        o_pool = ctx.enter_context(tc.tile_pool(name="o_pool", bufs=2))

        ident = const.tile([P, P], f32)
        make_identity(nc, ident)
        if dt_in != f32:
            ident_mm = const.tile([P, P], dt_in)
            make_identity(nc, ident_mm)
        else:
            ident_mm = ident

        # diagonal-pair causal masks (0/1), one per pair offset j0 in {0, 2}
        # elem[x, i1, y] = 1.0 iff y >= x + 128*(j0+i1)
        dmasks = []
        for j0 in (0, 2):
            m = const.tile([P, 2, CHUNK], dt_in, tag=f"dmask{j0}")
            nc.gpsimd.memset(m[:], 1.0)
            nc.gpsimd.affine_select(
                out=m[:],
                in_=m[:],
                compare_op=mybir.AluOpType.is_ge,
                fill=0.0,
                base=-j0 * P,
                pattern=[[-P, 2], [1, CHUNK]],
                channel_multiplier=-1,
            )
            dmasks.append(m)

        # --- HAM warmup: keep the PE busy with dummy matmuls while the
        # first xT chunk is in flight, so real matmuls start at full clock
        zeros_sb = const.tile([P, CHUNK], dt_in, tag="zeros")
        nc.vector.memset(zeros_sb[:], 0.0)
        dummy_ps = ot_ps.tile([P, CHUNK], f32, tag="ot")
        for _ in range(18):
            nc.tensor.matmul(
                dummy_ps,
                lhsT=zeros_sb[:, 0:P],
                rhs=zeros_sb[:],
                start=True,
                stop=True,
            )

        # --- weights: one rearranged DMA each, on the scalar HWDGE ring ---
        wqk_sb = const.tile([P, N_DT, 2 * E], dt_in, tag="wqk")
        nc.scalar.dma_start(
            wqk_sb[:], wqk_d.rearrange("(dt p) e -> p dt e", p=P)
        )
        wv_sb = const.tile([P, N_DT, E], dt_in, tag="wv")
        nc.scalar.dma_start(wv_sb[:], wv_d.rearrange("(dt p) e -> p dt e", p=P))

        # --- xT: one DMA per chunk on the sync ring (projection c0 can
        # start after the first ~0.8MB instead of the full 3MB) ---
        xT_sb = const.tile([P, N_DT, SEQ], dt_in, tag="xT")
        xT_re = xT_d.rearrange("(dt p) s -> p dt s", p=P)
        for c in range(N_CHUNKS):
            sl = slice(c * CHUNK, (c + 1) * CHUNK)
            nc.sync.dma_start(xT_sb[:, :, sl], xT_re[:, :, sl])

        # --- projections ---
        # A: partitions 0-63 = qT, 64-127 = kT;  B: the partition-swap of A
        A_sb = const.tile([P, SEQ], dt_in, tag="A")
        B_sb = const.tile([P, SEQ], dt_in, tag="B")
        vT_sb = const.tile([E, SEQ], dt_in, tag="vT")
        for c in range(N_CHUNKS):
            sl = slice(c * CHUNK, (c + 1) * CHUNK)
            ps = mm_ps.tile([P, 2, CHUNK], f32, tag="mm")
            for d in range(N_DT):
                nc.tensor.matmul(
                    ps[:, 0, :],
                    lhsT=wqk_sb[:, d, :],
                    rhs=xT_sb[:, d, sl],
                    start=(d == 0),
                    stop=(d == N_DT - 1),
                )
            nc.scalar.copy(A_sb[:, sl], ps[:, 0, :])
            # partition-shifted mirrors (validated on HW): kT -> base 0,
            # qT -> base 64
            nc.vector.tensor_copy(B_sb[0:E, sl], A_sb[E:P, sl])
            nc.vector.tensor_copy(B_sb[E:P, sl], A_sb[0:E, sl])

            psv = mm_ps.tile([E, 2, CHUNK], f32, tag="mm")
            for d in range(N_DT):
                nc.tensor.matmul(
                    psv[:, 0, :],
                    lhsT=wv_sb[:, d, :],
                    rhs=xT_sb[:, d, sl],
                    start=(d == 0),
                    stop=(d == N_DT - 1),
                )
            nc.vector.tensor_copy(vT_sb[:, sl], psv[:, 0, :])

        # --- v natural [128, 16, 65]: transpose vT back, ones in col E ---
        v_sb = const.tile([P, N_KT, E + 1], dt_in, tag="v")
        nc.vector.memset(v_sb[:], 1.0)
        for t in range(N_KT):
            vt = ot_ps.tile([P, E], dt_in, tag="ot")
            nc.tensor.transpose(
                vt, vT_sb[:, t * P:(t + 1) * P], ident_mm[0:E, 0:E]
            )
            nc.vector.tensor_copy(v_sb[:, t, 0:E], vt)

        # --- flash loop over q-chunks ---
        # Order c1,c2,c3,c0: c1 starts as soon as projections c0+c1 are
        # done (early exp start on ACT); the shortest chunk (c0) runs
        # last so the kernel tail is minimal. PV matmuls are delayed by
        # one pair so exp/mask latency never stalls the PE FIFO.
        pending = None  # (pv, v_t0, p2_0, v_t1, p2_1, start, stop)

        def flush_pending():
            nonlocal pending
            if pending is None:
                return
            pv_, l0, r0_, l1, r1, st_, sp_ = pending
            nc.tensor.matmul(pv_, lhsT=l0, rhs=r0_, start=st_, stop=False)
            nc.tensor.matmul(pv_, lhsT=l1, rhs=r1, start=False, stop=sp_)
            pending = None

        def finish_chunk(c, pv):
            # transpose [E+1, CHUNK] -> 4x [128, E+1], normalize, store
            pvT_sb = o_pool.tile([E + 1, CHUNK], f32, tag="pvT")
            nc.vector.tensor_copy(pvT_sb, pv)
            for st in range(CHUNK // P):
                ot = ot_ps.tile([P, E + 1], f32, tag="ot")
                nc.tensor.transpose(
                    ot,
                    pvT_sb[:, st * P:(st + 1) * P],
                    ident[0:E + 1, 0:E + 1],
                )
                recip = o_pool.tile([P, 1], f32, tag="recip")
                nc.vector.reciprocal(recip, ot[:, E:E + 1])
                o_sb = o_pool.tile([P, E], f32, tag="o")
                nc.vector.tensor_scalar_mul(o_sb, ot[:, 0:E], recip)
                r0 = c * CHUNK + st * P
                nc.sync.dma_start(out_d[r0:r0 + P, :], o_sb)

        finish_queue = []  # chunks whose PV accumulation is fully emitted
        for c in (1, 2, 3, 0):
            sl = slice(c * CHUNK, (c + 1) * CHUNK)
            n_pairs = 2 * (c + 1)  # causal: key tiles 0 .. 4c+3 in pairs
            pv = pv_ps.tile([E + 1, CHUNK], f32, tag="pv")
            for pi in range(n_pairs):
                t0, t1 = 2 * pi, 2 * pi + 1
                s2 = mm_ps.tile([P, 2, CHUNK], f32, tag="mm")
                # row-group (0,0): kT/qT from base partition 0
                nc.tensor.matmul(
                    s2[:, 0, :],
                    lhsT=B_sb[0:E, t0 * P:(t0 + 1) * P],
                    rhs=A_sb[0:E, sl],
                    start=True,
                    stop=True,
                )
                # row-group (64,0): kT/qT from base partition 64
                nc.tensor.matmul(
                    s2[:, 1, :],
                    lhsT=A_sb[E:P, t1 * P:(t1 + 1) * P],
                    rhs=B_sb[E:P, sl],
                    start=True,
                    stop=True,
                )
                flush_pending()
                if finish_queue:
                    # previous chunk's last PV just flushed; its pv psum
                    # can now be read and the slot released
                    finish_chunk(*finish_queue.pop(0))
                p2 = p_pool.tile([P, 2, CHUNK], dt_in, tag="p")
                nc.scalar.activation(
                    p2[:], s2[:], mybir.ActivationFunctionType.Exp,
                    scale=float(SCALE),
                )
                if t0 >= 4 * c:  # diagonal pair: zero entries with k > q
                    nc.vector.tensor_mul(
                        p2[:], p2[:], dmasks[(t0 - 4 * c) // 2][:]
                    )
                pending = (
                    pv,
                    v_sb[:, t0, :],
                    p2[:, 0, :],
                    v_sb[:, t1, :],
                    p2[:, 1, :],
                    pi == 0,
                    pi == n_pairs - 1,
                )
            finish_queue.append((c, pv))
        flush_pending()
        for item in finish_queue:
            finish_chunk(*item)

    nc.compile()
    return nc


def _get(dt_mm_name=None):
    name = dt_mm_name or DT_MM_NAME
    if name not in _CACHE:
        _CACHE[name] = _build(name)
    return _CACHE[name]


def _ensure_axon_hooks():
    """The agent image's antenv lacks axon_hooks; bass_utils imports it when
    trace=True under axon. Provide it, wired to the real ctypes NTFF
    profiler from trn_agent_boot when available."""
    try:
        import antenv.axon_hooks  # noqa: F401

        return
    except ImportError:
        pass
    import types

    try:
        import antenv
    except ImportError:
        return
    mod = types.ModuleType("antenv.axon_hooks")
    mod._hook = None

    def set_axon_ntff_profile_hook(h):
        mod._hook = h

    def get_axon_ntff_profile_hook():
        return mod._hook

    mod.set_axon_ntff_profile_hook = set_axon_ntff_profile_hook
    mod.get_axon_ntff_profile_hook = get_axon_ntff_profile_hook
    sys.modules["antenv.axon_hooks"] = mod
    antenv.axon_hooks = mod
    try:
        from trn_agent_boot.trn_boot import _ntff_profile_via_ctypes

        so_path = "/opt/axon/libaxon_pjrt.so"
        if os.path.exists(so_path):
            mod._hook = _ntff_profile_via_ctypes(so_path)
    except Exception:
        pass


def kernel(x, mask, Wq, Wk, Wv):
    global LAST_RESULT
    _ensure_axon_hooks()
    from concourse.bass_utils import run_bass_kernel_spmd

    nc = _get()

    if DT_MM_NAME == "bfloat16":
        import ml_dtypes

        np_dt = ml_dtypes.bfloat16
    else:
        np_dt = np.float32

    x = np.asarray(x, dtype=np.float32)
    wqk = np.concatenate(
        [np.asarray(Wq, dtype=np.float32), np.asarray(Wk, dtype=np.float32)],
        axis=1,
    ).astype(np_dt)
    wv = np.ascontiguousarray(np.asarray(Wv, dtype=np.float32)).astype(np_dt)

    in_maps = []
    for b in range(BS):
        in_maps.append(
            {
                "xT": np.ascontiguousarray(x[b].T).astype(np_dt),
                "Wqk": wqk,
                "Wv": wv,
            }
        )

    res = run_bass_kernel_spmd(nc, in_maps, core_ids=list(range(BS)))
    LAST_RESULT = res
    out = np.stack(
        [np.asarray(res.results[b]["out"], dtype=np.float32) for b in range(BS)],
        axis=0,
    )
    return out
